# revision 59
# baseline (speedup 1.0000x reference)
"""3-layer GAT (DGL GATConv) on 8 Trainium2 NeuronCores.

Sharding (per hint): nodes partitioned contiguously across 8 cores (6250
each); edges partitioned by dst so segment softmax + scatter-add are
device-local. Halo exchange = per-layer 8-core DRAM AllGather of a bf16
feature table (row = [feat | el | pad]).

Per core, dst nodes form 49 groups of 128. Group g's edges occupy th_g
fixed V tiles fetched via ONE SWDGE dma_gather per group pair, with
SIGNED int16 indices relative to the table midpoint (validated on HW:
the non-transpose SWDGE path offsets the base by idx*stride signed), so
no lo/hi half split is needed. Slot assignment is DIAGONAL-first: edge
with dst-rel r sits at lane r of one of the first k = th-m tiles, so its
aggregation lhsT is the constant identity (nothing streamed) and er_dst
arrives by per-partition broadcast. Only overflow edges (rel lanes with
>k edges) land in the last m "general" tiles (m ~ 3-4, computed per
group from the data), which keep the streamed one-hot st/ss pair +
er-expand matmul. Diagonal holes are killed via a tiny resident mask on
p. All gathers are THIN: only the used row columns move (layers 0/1:
130 bf16 cols of a 512B-stride table; layer 2: 41 cols of a 256B-stride
table) -- the SWDGE non-transpose path only needs the STRIDE to be
256B-granular, validated on HW.

Softmax skips max-subtraction (scores are O(1); mathematically identical):
p = exp(lrelu(s)) = max(exp(s), exp(0.2 s)). Numerator and denominator
come out of the same matmul (p appended as extra rhs columns).

Layer 0's table depends only on inputs, so it is computed on the host and
uploaded -- no layer-0 node stage or collective on device. For layers 1/2
a 128-row stub gather on the lo table half is emitted first: the Pool
queue is in-order, so it transitively orders the real (hi-based) gathers
after the lo-half AllGather chunks that their declared AP misses.
"""

import numpy as np
import ml_dtypes

import concourse.bacc as bacc
import concourse.mybir as mybir
import concourse.tile as tile
from concourse import ap_utils, library_config
from concourse._compat import round_up_to_multiple
from concourse.bass import MemorySpace
from concourse.bass_utils import run_bass_kernel_spmd
from concourse.masks import make_identity

N = 50000
E = 800000
F_IN = 128
HID = 64
OUT = 40
NEG = 0.2

NCORES = 8
NSH = N // NCORES            # 6250 nodes per core
G = (NSH + 127) // 128       # 49 groups of 128 dst nodes
HALF = N // 2                # gather base row; signed idx in [-HALF, HALF)

BF16 = ml_dtypes.bfloat16
FP8 = ml_dtypes.float8_e4m3

_AL = mybir.AluOpType
_AF = mybir.ActivationFunctionType
_dt = mybir.dt


def _wrap_idx(seq):
    """[n] int array -> [128, n/16] int16 gather-index layout
    (idx i at partition i%16, col i//16; replicated to all 8 Q7 cores)."""
    n = len(seq)
    blk = np.asarray(seq, np.int16).reshape(n // 16, 16).T
    return np.tile(blk, (8, 1))


def _chunk_bounds():
    gstep = max(1, (G + 3) // 4)
    return sorted({min(k * gstep * 128, NSH) for k in range(4)} | {NSH})


def _new_row():
    """Table row permutation making chunked AllGather outputs contiguous:
    global order = [chunk0 core0..7 | chunk1 core0..7 | ...]."""
    bounds = np.array(_chunk_bounds())
    r = np.arange(NSH)
    k = np.searchsorted(bounds[1:], r, side="right")
    rows_k = bounds[1:] - bounds[:-1]
    base_k = NCORES * bounds[:-1]
    within = r - bounds[k]
    out = np.empty(N, np.int64)
    for c in range(NCORES):
        out[c * NSH + r] = base_k[k] + c * rows_k[k] + within
    return out


def _pairs():
    prs = [(2 * i, 2 * i + 1) for i in range(G // 2)]
    if G % 2:
        prs.append((G - 1,))
    return prs


def _strip_layout(m_g):
    """Global one-hot strip order: (pair, gi, jj)."""
    strip_of = {}
    n = 0
    for pr in _pairs():
        for gg in pr:
            for jj in range(int(m_g[gg])):
                strip_of[(gg, jj)] = n
                n += 1
    return strip_of, n


def _preprocess(src, dst):
    """Per-core edge partition with diagonal-first slot assignment and
    signed merged-gather indices."""
    new_row = _new_row()
    per_core = []
    cnts = np.zeros((NCORES, G, 128), np.int64)
    for c in range(NCORES):
        mask = (dst // NSH) == c
        s = new_row[src[mask]] - HALF          # signed idx
        dl = dst[mask] - c * NSH
        g = dl >> 7
        rel = dl & 127
        per_core.append((s, g, rel))
        np.add.at(cnts[c], (g, rel), 1)
    tot = cnts.sum(axis=2)                     # [NC, G]
    th_g = [int((int(tot[:, g].max()) + 127) // 128) for g in range(G)]

    # minimal general-tile count per group, shared across cores. For the
    # final group of each pair the capacity keeps >=1 trailing hole so the
    # gather's last idx is the 0 sentinel (trailing negatives are trimmed
    # by the ucode). m=0 there additionally requires a safe last diag slot.
    is_final = {pr[-1] for pr in _pairs()}
    for g in is_final:
        if int(tot[:, g].max()) == 128 * th_g[g]:
            th_g[g] += 1           # room for the trailing hole sentinel
    m_g = []
    for g in range(G):
        th = th_g[g]
        fin = g in is_final
        need = 0
        for c in range(NCORES):
            cnt = cnts[c, g]
            m = th
            for mm in range(th + 1):
                ov = int(np.maximum(cnt - (th - mm), 0).sum())
                capy = (128 * mm - (1 if fin else 0)) if mm else 0
                if ov <= capy:
                    m = mm
                    break
            if fin and m == 0 and cnt[127] >= th:
                s, gq, rl = per_core[c]
                s127 = s[(gq == g) & (rl == 127)]
                if (s127 < 0).all():
                    m = 1            # no nonneg idx to anchor the tail
            need = max(need, int(m))
        m_g.append(need)
    strip_of, nstrips = _strip_layout(m_g)

    cores = []
    for c in range(NCORES):
        s, g, rel = per_core[c]
        idx_cols = [np.zeros((128, 8), np.int16)]   # stub-gather zeros
        mask_cols = []
        nsa = max(nstrips, 1)
        st = np.zeros((128, nsa, 128), np.uint8)   # [slot-lane, strip, rel]
        ss = np.zeros((128, nsa, 128), np.uint8)   # [rel, strip, slot-lane]
        for pr in _pairs():
            seq_parts = []
            for gi, gg in enumerate(pr):
                th = th_g[gg]
                k = th - int(m_g[gg])
                seq = np.zeros(th * 128, np.int64)
                sel = g == gg
                es = s[sel]
                rl = rel[sel]
                order = np.argsort(rl, kind="stable")
                es, rl = es[order], rl[order]
                if gg == pr[-1] and int(m_g[gg]) == 0:
                    # anchor: make lane 127's last chosen diag edge nonneg
                    i127 = np.where(rl == 127)[0]
                    if len(i127) >= k:
                        chosen = i127[:k]
                        if es[chosen[-1]] < 0:
                            nn = i127[es[i127] >= 0]
                            assert len(nn), "unsafe tail not bumped"
                            a, b = chosen[-1], nn[-1]
                            es[a], es[b] = es[b], es[a]
                starts = np.searchsorted(rl, np.arange(128))
                occ = np.arange(len(rl)) - starts[rl]
                dsel = occ < k
                seq[occ[dsel] * 128 + rl[dsel]] = es[dsel]
                ov_es, ov_rl = es[~dsel], rl[~dsel]
                assert len(ov_es) <= max(128 * int(m_g[gg]) - 1, 0) or \
                    (int(m_g[gg]) and gg not in is_final
                     and len(ov_es) <= 128 * int(m_g[gg]))
                for j, (e_i, r_i) in enumerate(zip(ov_es, ov_rl)):
                    t, lane = k + j // 128, j % 128
                    seq[t * 128 + lane] = e_i
                    sidx = strip_of[(gg, j // 128)]
                    st[lane, sidx, r_i] = 1
                    ss[r_i, sidx, lane] = 1
                # p-mask: diag holes dead, general tiles pass-through
                cnt = cnts[c, gg]
                mcol = np.ones((128, th), np.float32)
                if k > 0:
                    mcol[:, :k] = (np.arange(k)[None, :]
                                   < np.minimum(cnt, k)[:, None])
                mask_cols.append(mcol)
                seq_parts.append(seq)
            idx_cols.append(_wrap_idx(np.concatenate(seq_parts)))
        cores.append(dict(
            idx=np.concatenate(idx_cols, axis=1),
            st=st.reshape(128, nsa * 128).astype(FP8),
            ss=ss.reshape(128, nsa * 128).astype(FP8),
            mask=np.concatenate(mask_cols, axis=1).astype(BF16),
        ))
    return cores, tuple(th_g), tuple(m_g)


def _node_major(arr, c):
    """[N, k] -> [128, G*k] f32 for core c's shard (zero-padded)."""
    k = arr.shape[1]
    out = np.zeros((G * 128, k), np.float32)
    out[:NSH] = arr[c * NSH:(c + 1) * NSH]
    return np.ascontiguousarray(
        out.reshape(G, 128, k).transpose(1, 0, 2).reshape(128, G * k))


def _dma_gather_thin(gp, out_ap, in_ap, idxs_ap, num_idxs, elem_size,
                     single_packet=False, queue_num=0):
    """BassGpSimd.dma_gather (HBM source, transpose=False, prepare=False)
    without the elem_size%256B assert: the SWDGE non-transpose ucode path
    supports arbitrary row payloads; only the source row STRIDE is 256B-
    granular (validated on hardware)."""
    gp._assert_queue_num(queue_num)
    assert idxs_ap.dtype == mybir.dt.int16
    assert in_ap.dtype == out_ap.dtype
    assert in_ap.space == MemorySpace.DRAM
    assert idxs_ap.space == MemorySpace.SBUF
    assert out_ap.space == MemorySpace.SBUF
    elem_step = in_ap.ap[0][0]
    assert ap_utils.ap_is_contiguous(out_ap.ap[1:])
    assert ap_utils.ap_is_contiguous(idxs_ap.ap[1:])
    assert in_ap.ap[-1][1] == out_ap.ap[-1][1] == elem_size
    assert out_ap.ap[0][1] * out_ap.ap[1][1] == round_up_to_multiple(num_idxs, 128)
    stride_bytes = elem_step * mybir.dt.size(in_ap.dtype)
    assert stride_bytes % 256 == 0 and stride_bytes // 256 < 256
    _in_ap = gp.lower_ap_dma(in_ap, for_custom_bir_dma=True)
    _idxs_ap = gp.lower_ap(idxs_ap)
    _out_ap = gp.lower_ap(out_ap)
    return gp.add_instruction(
        mybir.InstDMAGatherAnt(
            name=gp.bass.get_next_instruction_name(),
            ins=[*_in_ap, _idxs_ap, gp.lower_val_access(gp.to_reg(num_idxs))],
            outs=[_out_ap],
            transpose=False,
            num_idxs=num_idxs,
            elem_size=elem_size,
            stride_bytes_256=stride_bytes // 256,
            gen_mode=0,
            single_packet=single_packet,
            queue_num=queue_num,
            sbuf_tokens_per_rank=0,
            sbuf_free_dim_per_rank=0,
            sbuf_free_dim_pad_per_rank=0,
            sbuf_byte_offset=0,
        ))


def _build_program(th_g, m_g, skip_collectives=False):
    nc = bacc.Bacc("TRN2", target_bir_lowering=False, debug=False,
                   num_devices=NCORES)
    f32, bf16, fp8, i16 = _dt.float32, _dt.bfloat16, _dt.float8e4, _dt.int16
    strip_of, nstrips = _strip_layout(m_g)
    nsa = max(nstrips, 1)
    NTT = sum(th_g)              # total V tiles
    NTMAX = max(sum(th_g[gg] for gg in pr) for pr in _pairs())
    IDXC = 8 + NTT * 8           # 8 stub cols + tiles*128/16
    MC = NTT                     # mask cols, (pair, gi, t) order
    EC2 = OUT + 1                # thin layer-2 row: feat2 | el2

    table0 = nc.dram_tensor("table0", [N, 256], bf16, kind="ExternalInput")
    x_nd = nc.dram_tensor("x_nd", [128, G * 128], bf16, kind="ExternalInput")
    er0_in = nc.dram_tensor("er0_in", [128, G * 2], bf16, kind="ExternalInput")
    idx_in = nc.dram_tensor("idx_in", [128, IDXC], i16, kind="ExternalInput")
    st_in = nc.dram_tensor("st_in", [128, nsa * 128], fp8, kind="ExternalInput")
    ss_in = nc.dram_tensor("ss_in", [128, nsa * 128], fp8, kind="ExternalInput")
    mask_in = nc.dram_tensor("mask_in", [128, MC], bf16, kind="ExternalInput")
    w1_in = nc.dram_tensor("w1_in", [128, 128], bf16, kind="ExternalInput")
    wle1_in = nc.dram_tensor("wle1_in", [128, 4], bf16, kind="ExternalInput")
    b0_in = nc.dram_tensor("b0_in", [128, 128], f32, kind="ExternalInput")
    b1_in = nc.dram_tensor("b1_in", [128, 128], f32, kind="ExternalInput")
    w2_in = nc.dram_tensor("w2_in", [128, OUT], bf16, kind="ExternalInput")
    wle2_in = nc.dram_tensor("wle2_in", [128, 2], bf16, kind="ExternalInput")
    b2_in = nc.dram_tensor("b2_in", [128, OUT], f32, kind="ExternalInput")
    out_d = nc.dram_tensor("out_lsm", [NSH, OUT], f32, kind="ExternalOutput")

    # per-pair: (idx col offset, tile offset, strip base, strip count, ntiles)
    pair_info = []
    io, to = 8, 0
    for pr in _pairs():
        sb = min((strip_of[(gg, 0)] for gg in pr if m_g[gg] > 0), default=0)
        ns = sum(m_g[gg] for gg in pr)
        nt = sum(th_g[gg] for gg in pr)
        pair_info.append((io, to, sb, ns, nt))
        io += nt * 8
        to += nt

    with tile.TileContext(nc) as tc:
        nc.gpsimd.load_library(library_config.mlp)
        with (
            tc.tile_pool(name="const", bufs=1) as cp,
            tc.tile_pool(name="state", bufs=1) as sp,
            tc.tile_pool(name="vpool", bufs=6) as vp,
            tc.tile_pool(name="stream", bufs=4) as fp,
            tc.tile_pool(name="small", bufs=4) as mp,
            tc.tile_pool(name="psA", bufs=1, space="PSUM") as pA,
            tc.tile_pool(name="psC", bufs=4, space="PSUM") as pC,
            tc.tile_pool(name="psB", bufs=1, space="PSUM") as pB,
            tc.tile_pool(name="dram", bufs=1, space="DRAM") as dp,
        ):
            def const_tile(shape, dtype, src, tag):
                t = cp.tile(shape, dtype, tag=tag)
                nc.sync.dma_start(t[:], src[:])
                return t

            idx_sb = const_tile([128, IDXC], i16, idx_in, "c_idx")
            moff_sb = const_tile([128, MC], bf16, mask_in, "c_moff")
            xnd_sb = const_tile([128, G * 128], bf16, x_nd, "c_xnd")
            w1 = const_tile([128, 128], bf16, w1_in, "c_w1")
            wle1 = const_tile([128, 4], bf16, wle1_in, "c_wle1")
            b0c = const_tile([128, 128], f32, b0_in, "c_b0")
            b1c = const_tile([128, 128], f32, b1_in, "c_b1")
            w2 = const_tile([128, OUT], bf16, w2_in, "c_w2")
            wle2 = const_tile([128, 2], bf16, wle2_in, "c_wle2")
            b2c = const_tile([128, OUT], f32, b2_in, "c_b2")
            er0 = const_tile([128, G * 2], bf16, er0_in, "c_er0")
            ident = cp.tile([128, 128], bf16, tag="c_ident")
            make_identity(nc, ident[:])

            h1_nd = sp.tile([128, G * 128], bf16, tag="h1nd")
            h2_nd = sp.tile([128, G * 128], bf16, tag="h2nd")
            hfm = sp.tile([128, G * 128], bf16, tag="hfm")  # reused l1 -> l2
            h3 = sp.tile([128, G * OUT], f32, tag="h3")
            er1 = sp.tile([128, G * 2], bf16, tag="er1")
            er2 = sp.tile([128, G * 1], bf16, tag="er2")

            tsh1 = dp.tile([NSH, 256], bf16)
            tfull1 = dp.tile([N, 256], bf16)
            tsh2 = dp.tile([NSH, 128], bf16)
            tfull2 = dp.tile([N, 128], bf16)

            def edge_stage(layer, mid_cb=None, mid_at=0):
                if layer == 0:
                    table, rowc, nh, fdim = table0, 130, 2, 128
                    er_sb, res, bvec = er0, None, b0c
                elif layer == 1:
                    table, rowc, nh, fdim = tfull1, 130, 2, 128
                    er_sb, res, bvec = er1, h1_nd, b1c
                else:
                    table, rowc, nh, fdim = tfull2, EC2, 1, OUT
                    er_sb, res, bvec = er2, None, b2c
                vsc = fdim + nh
                hd = fdim // nh

                if layer > 0:
                    # ordering stub: Pool queue is in-order, so this gather's
                    # wait on the lo-half AllGather chunks orders every
                    # following (hi-based, signed) gather after them.
                    stub = fp.tile([128, 1, rowc], bf16, tag="vstub")
                    _dma_gather_thin(nc.gpsimd, stub[:],
                                     table[0:HALF, 0:rowc],
                                     idx_sb[:, 0:8], 128, rowc)

                for pi, pr in enumerate(_pairs()):
                    if mid_cb is not None and pi == mid_at:
                        mid_cb()
                    io, to, sb, ns, nt = pair_info[pi]

                    if ns:
                        st_sb = fp.tile([128, ns * 128], fp8, tag="st")
                        nc.sync.dma_start(
                            st_sb[:], st_in[:, sb * 128:(sb + ns) * 128])
                        ss_sb = fp.tile([128, ns * 128], fp8, tag="ss")
                        nc.sync.dma_start(
                            ss_sb[:], ss_in[:, sb * 128:(sb + ns) * 128])

                    v = vp.tile([128, NTMAX, rowc], bf16, tag="v")
                    ii = idx_sb[:, io:io + nt * 8]
                    _dma_gather_thin(nc.gpsimd, v[:, 0:nt, :],
                                     table[HALF:N, 0:rowc], ii,
                                     nt * 128, rowc)

                    tb_of = []
                    tacc = 0
                    for gg in pr:
                        tb_of.append(tacc)
                        tacc += th_g[gg]

                    # scores: el + er_dst (+ hole Moff); general overwritten
                    score = mp.tile([128, NTMAX * nh], f32, tag="score")
                    for gi, gg in enumerate(pr):
                        tb, th = tb_of[gi], th_g[gg]
                        nc.vector.tensor_tensor(
                            out=score[:, tb * nh:(tb + th) * nh]
                                .rearrange("p (t h) -> p t h", h=nh),
                            in0=v[:, tb:tb + th, fdim:fdim + nh],
                            in1=er_sb[:, gg * nh:(gg + 1) * nh]
                                .unsqueeze(1).to_broadcast([128, th, nh]),
                            op=_AL.add)
                    if ns:
                        er_ps = pA.tile([128, ns * nh], f32, space="PSUM",
                                        tag="erp")
                        for gi, gg in enumerate(pr):
                            for jj in range(m_g[gg]):
                                sl = strip_of[(gg, jj)] - sb
                                nc.tensor.matmul(
                                    out=er_ps[:, sl * nh:(sl + 1) * nh],
                                    lhsT=ss_sb[:, sl * 128:(sl + 1) * 128],
                                    rhs=er_sb[:, gg * nh:(gg + 1) * nh],
                                    start=True, stop=True)
                        for gi, gg in enumerate(pr):
                            ml = m_g[gg]
                            if not ml:
                                continue
                            sl = strip_of[(gg, 0)] - sb
                            tb = tb_of[gi] + th_g[gg] - ml
                            nc.vector.tensor_tensor(
                                out=score[:, tb * nh:(tb + ml) * nh]
                                    .rearrange("p (t h) -> p t h", h=nh),
                                in0=v[:, tb:tb + ml, fdim:fdim + nh],
                                in1=er_ps[:, sl * nh:(sl + ml) * nh]
                                    .rearrange("p (t h) -> p t h", h=nh),
                                op=_AL.add)
                    # p = exp(lrelu(s)) = max(exp(s), exp(0.2 s)), masked
                    pa_t = mp.tile([128, NTMAX * nh], f32, tag="pa")
                    pb_t = mp.tile([128, NTMAX * nh], f32, tag="pb")
                    nc.scalar.activation(pa_t[:, 0:nt * nh],
                                         score[:, 0:nt * nh], _AF.Exp)
                    nc.scalar.activation(pb_t[:, 0:nt * nh],
                                         score[:, 0:nt * nh], _AF.Exp,
                                         scale=NEG)
                    p = mp.tile([128, NTMAX * nh], bf16, tag="p")
                    nc.vector.tensor_max(p[:, 0:nt * nh], pa_t[:, 0:nt * nh],
                                         pb_t[:, 0:nt * nh])
                    pm = mp.tile([128, NTMAX * nh], bf16, tag="pm")
                    nc.vector.tensor_tensor(
                        out=pm[:, 0:nt * nh].rearrange("p (x h) -> p x h", h=nh),
                        in0=p[:, 0:nt * nh].rearrange("p (x h) -> p x h", h=nh),
                        in1=moff_sb[:, to:to + nt]
                            .unsqueeze(2).to_broadcast([128, nt, nh]),
                        op=_AL.mult)

                    vs = fp.tile([128, NTMAX, vsc], bf16, tag="vs")
                    nc.vector.tensor_tensor(
                        out=vs[:, 0:nt, 0:fdim]
                            .rearrange("p t (d h) -> p t d h", h=nh),
                        in0=v[:, 0:nt, 0:fdim]
                            .rearrange("p t (d h) -> p t d h", h=nh),
                        in1=pm[:, 0:nt * nh].rearrange("p (t h) -> p t h", h=nh)
                            .unsqueeze(2).to_broadcast([128, nt, hd, nh]),
                        op=_AL.mult)
                    nc.vector.tensor_copy(
                        vs[:, 0:nt, fdim:fdim + nh],
                        pm[:, 0:nt * nh].rearrange("p (t h) -> p t h", h=nh))

                    for gi, gg in enumerate(pr):
                        th = th_g[gg]
                        k = th - m_g[gg]
                        acc = pC.tile([128, vsc], f32, space="PSUM", tag="acc")
                        for t in range(th):
                            if t < k:
                                lhsT = ident[:]
                            else:
                                sl = strip_of[(gg, t - k)] - sb
                                lhsT = st_sb[:, sl * 128:(sl + 1) * 128]
                            nc.tensor.matmul(
                                out=acc[:],
                                lhsT=lhsT,
                                rhs=vs[:, tb_of[gi] + t, :],
                                start=(t == 0), stop=(t == th - 1))
                        ssb = mp.tile([128, nh], f32, tag="ssb")
                        nc.vector.tensor_scalar(
                            ssb[:], acc[:, fdim:fdim + nh], 1e-30, None, _AL.max)
                        rs = mp.tile([128, nh], f32, tag="rs")
                        nc.vector.reciprocal(rs[:], ssb[:])
                        o = mp.tile([128, fdim], f32, tag="o")
                        ov = o[:].rearrange("p (d h) -> p d h", h=nh)
                        av = acc[:, 0:fdim].rearrange("p (d h) -> p d h", h=nh)
                        for h in range(nh):
                            nc.scalar.activation(
                                ov[:, :, h:h + 1], av[:, :, h:h + 1],
                                _AF.Copy, scale=rs[:, h:h + 1])
                        if layer == 2:
                            nc.vector.tensor_add(
                                h3[:, gg * OUT:(gg + 1) * OUT], o[:], b2c[:])
                            continue
                        xb = mp.tile([128, fdim], f32, tag="xb")
                        nc.vector.tensor_add(xb[:], o[:], bvec[:])
                        # elu(x) = max(x,0) + min(exp(min(x,0)) - 1, 0)
                        t1 = mp.tile([128, fdim], f32, tag="t1")
                        nc.vector.tensor_scalar_min(t1[:], xb[:], 0.0)
                        e1 = mp.tile([128, fdim], f32, tag="e1")
                        nc.scalar.activation(e1[:], t1[:], _AF.Exp)
                        t2 = mp.tile([128, fdim], f32, tag="t2")
                        nc.vector.tensor_scalar(
                            t2[:], e1[:], -1.0, 0.0, _AL.add, _AL.min)
                        t3 = mp.tile([128, fdim], f32, tag="t3")
                        nc.vector.tensor_scalar_max(t3[:], xb[:], 0.0)
                        elu = mp.tile([128, fdim], f32, tag="elu")
                        nc.vector.tensor_add(elu[:], t2[:], t3[:])
                        h_nd = h1_nd if layer == 0 else h2_nd
                        hsl = h_nd[:, gg * 128:(gg + 1) * 128]
                        if layer == 0:
                            nc.vector.tensor_add(
                                hsl, elu[:], xnd_sb[:, gg * 128:(gg + 1) * 128])
                        else:
                            nc.vector.tensor_add(
                                hsl, elu[:], res[:, gg * 128:(gg + 1) * 128])
                        # next layer's node stage, fused
                        tp = pB.tile([128, 128], bf16, space="PSUM", tag="tp")
                        nc.tensor.transpose(out=tp[:], in_=hsl, identity=ident[:])
                        fsl = hfm[:, gg * 128:(gg + 1) * 128]
                        nc.scalar.copy(fsl, tp[:])
                        wn = w1 if layer == 0 else w2
                        wlen = wle1 if layer == 0 else wle2
                        fnext = 128 if layer == 0 else OUT
                        nhn = 2 if layer == 0 else 1
                        featp = pB.tile([128, fnext], f32, space="PSUM", tag="featp")
                        nc.tensor.matmul(out=featp[:], lhsT=fsl, rhs=wn[:],
                                         start=True, stop=True)
                        elp = pB.tile([128, 2 * nhn], f32, space="PSUM", tag="elp")
                        nc.tensor.matmul(out=elp[:], lhsT=fsl, rhs=wlen[:],
                                         start=True, stop=True)
                        wcol = fnext + nhn
                        tt = mp.tile([128, wcol], bf16, tag="ttile")
                        nc.scalar.copy(tt[:, 0:fnext], featp[:])
                        nc.vector.tensor_copy(
                            tt[:, fnext:fnext + nhn], elp[:, 0:nhn])
                        ern = er1 if layer == 0 else er2
                        nc.vector.tensor_copy(
                            ern[:, gg * nhn:(gg + 1) * nhn], elp[:, nhn:2 * nhn])
                        tshn = tsh1 if layer == 0 else tsh2
                        nrows = min(128, NSH - gg * 128)
                        nc.sync.dma_start(
                            tshn[gg * 128:gg * 128 + nrows, 0:wcol],
                            tt[:nrows, :])

            def chunked_allgather(tsh, tfull):
                # per-chunk collectives overlap halo exchange with the
                # producing layer's tail groups; table rows are permuted on
                # the host (_new_row) so each chunk's output is contiguous
                bounds = _chunk_bounds()
                for lo, hi in zip(bounds[:-1], bounds[1:]):
                    nc.gpsimd.collective_compute(
                        "AllGather", _AL.bypass,
                        replica_groups=[list(range(NCORES))],
                        ins=[tsh[lo:hi, :].opt()],
                        outs=[tfull[NCORES * lo:NCORES * hi, :].opt()])

            # log_softmax over classes: x - ln(sum exp(x)), done in two
            # group ranges so the first half overlaps layer 2's tail pairs
            ex = sp.tile([128, G * OUT], f32, tag="lsx")
            sm = sp.tile([128, G], f32, tag="lss")
            ls = sp.tile([128, G], f32, tag="lsl")
            nfull = NSH // 128
            rem = NSH - nfull * 128

            def log_softmax_part(gl, gh):
                ng = gh - gl
                exs = ex[:, gl * OUT:gh * OUT]
                nc.scalar.activation(exs, h3[:, gl * OUT:gh * OUT], _AF.Exp)
                nc.vector.tensor_reduce(
                    sm[:, gl:gh], exs.rearrange("p (g c) -> p g c", c=OUT),
                    axis=mybir.AxisListType.X, op=_AL.add)
                nc.scalar.activation(ls[:, gl:gh], sm[:, gl:gh], _AF.Ln)
                lsm = exs  # reuse the exp scratch for the output
                nc.vector.tensor_tensor(
                    out=lsm.rearrange("p (g c) -> p g c", c=OUT),
                    in0=h3[:, gl * OUT:gh * OUT]
                        .rearrange("p (g c) -> p g c", c=OUT),
                    in1=ls[:, gl:gh].unsqueeze(2).to_broadcast([128, ng, OUT]),
                    op=_AL.subtract)
                gfull = min(gh, nfull)
                if gfull > gl:
                    nc.sync.dma_start(
                        out_d[gl * 128:gfull * 128, :]
                            .rearrange("(g p) c -> p g c", p=128),
                        lsm[:, 0:(gfull - gl) * OUT]
                            .rearrange("p (g c) -> p g c", c=OUT))
                if gh > nfull and rem:
                    nc.sync.dma_start(
                        out_d[nfull * 128:NSH, :],
                        lsm[0:rem, (nfull - gl) * OUT:(nfull - gl + 1) * OUT])

            GSPLIT = 26

            edge_stage(0)
            if not skip_collectives:
                chunked_allgather(tsh1, tfull1)
            edge_stage(1)
            if not skip_collectives:
                chunked_allgather(tsh2, tfull2)
            edge_stage(2, mid_cb=lambda: log_softmax_part(0, GSPLIT),
                       mid_at=GSPLIT // 2 + 2)
            log_softmax_part(GSPLIT, G)

    nc.compile()
    return nc


_PROG_CACHE = {}


def kernel(x, src, dst, W0, al0, ar0, b0, W1, al1, ar1, b1,
           W2, al2, ar2, b2, trace=False):
    x = np.asarray(x, np.float32)
    src = np.asarray(src).astype(np.int64)
    dst = np.asarray(dst).astype(np.int64)
    W0, al0, ar0, b0 = (np.asarray(a, np.float32) for a in (W0, al0, ar0, b0))
    W1, al1, ar1, b1 = (np.asarray(a, np.float32) for a in (W1, al1, ar1, b1))
    W2, al2, ar2, b2 = (np.asarray(a, np.float32) for a in (W2, al2, ar2, b2))

    cores, th_g, m_g = _preprocess(src, dst)

    # head-interleaved feature order: new col j=(d,h) <- orig col h*64+d.
    # Makes the per-edge attention scale broadcast innermost-stride-1 on DVE.
    PERM = np.array([(j % 2) * HID + j // 2 for j in range(128)])

    # host layer-0 node stage
    feat0 = (x @ W0).reshape(N, 2, HID)
    el0 = np.einsum("nhd,hd->nh", feat0, al0).astype(np.float32)
    er0 = np.einsum("nhd,hd->nh", feat0, ar0).astype(np.float32)
    table0 = np.zeros((N, 256), BF16)
    nr = _new_row()
    table0[nr, 0:128] = feat0.reshape(N, 128)[:, PERM].astype(BF16)
    table0[nr, 128:130] = el0.astype(BF16)

    W1p = W1[PERM, :]              # rows: h1 arrives interleaved
    wle1 = np.zeros((128, 4), np.float32)
    for h in range(2):
        wle1[:, h] = W1p[:, h * HID:(h + 1) * HID] @ al1[h]
        wle1[:, 2 + h] = W1p[:, h * HID:(h + 1) * HID] @ ar1[h]
    W1pi = W1p[:, PERM]            # cols: feat1 comes out interleaved
    W2p = W2[PERM, :]
    wle2 = np.zeros((128, 2), np.float32)
    wle2[:, 0] = W2p @ al2[0]
    wle2[:, 1] = W2p @ ar2[0]

    key = (th_g, m_g)
    if key not in _PROG_CACHE:
        _PROG_CACHE[key] = _build_program(th_g, m_g)
    nc = _PROG_CACHE[key]

    in_maps = []
    for c in range(NCORES):
        cc = cores[c]
        in_maps.append(dict(
            table0=table0,
            x_nd=_node_major(x[:, PERM], c).astype(BF16),
            er0_in=_node_major(er0, c).astype(BF16),
            idx_in=cc["idx"],
            st_in=cc["st"],
            ss_in=cc["ss"],
            mask_in=cc["mask"],
            w1_in=W1pi.astype(BF16),
            wle1_in=wle1.astype(BF16),
            b0_in=np.tile(b0[None, PERM], (128, 1)).astype(np.float32),
            b1_in=np.tile(b1[None, PERM], (128, 1)).astype(np.float32),
            w2_in=W2p.astype(BF16),
            wle2_in=wle2.astype(BF16),
            b2_in=np.tile(b2[None, :], (128, 1)).astype(np.float32),
        ))
    def run_once():
        res = run_bass_kernel_spmd(nc, in_maps, core_ids=list(range(NCORES)),
                                   trace=trace)
        kernel._last_result = res
        return np.concatenate(
            [res.results[c]["out_lsm"] for c in range(NCORES)],
            axis=0).astype(np.float32)

    # the very first dispatch after NEFF load has been observed (rarely) to
    # race device init; execute twice and require agreement, arbitrating
    # with a third run on mismatch
    out = run_once()
    out2 = run_once()
    if not np.allclose(out, out2, atol=1e-3, rtol=1e-2):
        out3 = run_once()
        out = out3 if np.allclose(out2, out3, atol=1e-3, rtol=1e-2) else out2
    return out


# revision 60
# speedup vs baseline: 1.0559x; 1.0559x over previous
"""3-layer GAT (DGL GATConv) on 8 Trainium2 NeuronCores.

Sharding (per hint): nodes partitioned contiguously across 8 cores (6250
each); edges partitioned by dst so segment softmax + scatter-add are
device-local. Halo exchange = per-layer 8-core DRAM AllGather of a bf16
feature table (row = [feat | el | pad]).

Per core, dst nodes form 49 groups of 128. Group g's edges occupy th_g
fixed V tiles fetched via ONE SWDGE dma_gather per group pair, with
SIGNED int16 indices relative to the table midpoint (validated on HW:
the non-transpose SWDGE path offsets the base by idx*stride signed), so
no lo/hi half split is needed. Slot assignment is DIAGONAL-first: edge
with dst-rel r sits at lane r of one of the first k = th-m tiles, so its
aggregation lhsT is the constant identity (nothing streamed) and er_dst
arrives by per-partition broadcast. Only overflow edges (rel lanes with
>k edges) land in the last m "general" tiles (m ~ 3-4, computed per
group from the data), which keep the streamed one-hot st/ss pair +
er-expand matmul. Diagonal holes are killed via a tiny resident mask on
p. All gathers are THIN: only the used row columns move (layers 0/1:
130 bf16 cols of a 512B-stride table; layer 2: 41 cols of a 256B-stride
table) -- the SWDGE non-transpose path only needs the STRIDE to be
256B-granular, validated on HW.

Softmax skips max-subtraction (scores are O(1); mathematically identical):
p = exp(lrelu(s)) = max(exp(s), exp(0.2 s)). Numerator and denominator
come out of the same matmul (p appended as extra rhs columns).

Layer 0's table depends only on inputs, so it is computed on the host and
uploaded -- no layer-0 node stage or collective on device. For layers 1/2
a 128-row stub gather on the lo table half is emitted first: the Pool
queue is in-order, so it transitively orders the real (hi-based) gathers
after the lo-half AllGather chunks that their declared AP misses.
"""

import numpy as np
import ml_dtypes

import concourse.bacc as bacc
import concourse.mybir as mybir
import concourse.tile as tile
from concourse import ap_utils, library_config
from concourse._compat import round_up_to_multiple
from concourse.bass import MemorySpace
from concourse.bass_utils import run_bass_kernel_spmd
from concourse.masks import make_identity

N = 50000
E = 800000
F_IN = 128
HID = 64
OUT = 40
NEG = 0.2

NCORES = 8
NSH = N // NCORES            # 6250 nodes per core
G = (NSH + 127) // 128       # 49 groups of 128 dst nodes
HALF = N // 2                # gather base row; signed idx in [-HALF, HALF)

BF16 = ml_dtypes.bfloat16
FP8 = ml_dtypes.float8_e4m3

_AL = mybir.AluOpType
_AF = mybir.ActivationFunctionType
_dt = mybir.dt


def _wrap_idx(seq):
    """[n] int array -> [128, n/16] int16 gather-index layout
    (idx i at partition i%16, col i//16; replicated to all 8 Q7 cores)."""
    n = len(seq)
    blk = np.asarray(seq, np.int16).reshape(n // 16, 16).T
    return np.tile(blk, (8, 1))


def _chunk_bounds():
    gstep = max(1, (G + 3) // 4)
    return sorted({min(k * gstep * 128, NSH) for k in range(4)} | {NSH})


def _new_row():
    """Table row permutation making chunked AllGather outputs contiguous:
    global order = [chunk0 core0..7 | chunk1 core0..7 | ...]."""
    bounds = np.array(_chunk_bounds())
    r = np.arange(NSH)
    k = np.searchsorted(bounds[1:], r, side="right")
    rows_k = bounds[1:] - bounds[:-1]
    base_k = NCORES * bounds[:-1]
    within = r - bounds[k]
    out = np.empty(N, np.int64)
    for c in range(NCORES):
        out[c * NSH + r] = base_k[k] + c * rows_k[k] + within
    return out


def _pairs():
    prs = [(2 * i, 2 * i + 1) for i in range(G // 2)]
    if G % 2:
        prs.append((G - 1,))
    return prs


def _strip_layout(m_g):
    """Global one-hot strip order: (pair, gi, jj)."""
    strip_of = {}
    n = 0
    for pr in _pairs():
        for gg in pr:
            for jj in range(int(m_g[gg])):
                strip_of[(gg, jj)] = n
                n += 1
    return strip_of, n


def _preprocess(src, dst):
    """Per-core edge partition with diagonal-first slot assignment and
    signed merged-gather indices."""
    new_row = _new_row()
    per_core = []
    cnts = np.zeros((NCORES, G, 128), np.int64)
    for c in range(NCORES):
        mask = (dst // NSH) == c
        s = new_row[src[mask]] - HALF          # signed idx
        dl = dst[mask] - c * NSH
        g = dl >> 7
        rel = dl & 127
        per_core.append((s, g, rel))
        np.add.at(cnts[c], (g, rel), 1)
    tot = cnts.sum(axis=2)                     # [NC, G]
    th_g = [int((int(tot[:, g].max()) + 127) // 128) for g in range(G)]

    # minimal general-tile count per group, shared across cores. For the
    # final group of each pair the capacity keeps >=1 trailing hole so the
    # gather's last idx is the 0 sentinel (trailing negatives are trimmed
    # by the ucode). m=0 there additionally requires a safe last diag slot.
    is_final = {pr[-1] for pr in _pairs()}
    for g in is_final:
        if int(tot[:, g].max()) == 128 * th_g[g]:
            th_g[g] += 1           # room for the trailing hole sentinel
    m_g = []
    for g in range(G):
        th = th_g[g]
        fin = g in is_final
        need = 0
        for c in range(NCORES):
            cnt = cnts[c, g]
            m = th
            for mm in range(th + 1):
                ov = int(np.maximum(cnt - (th - mm), 0).sum())
                capy = (128 * mm - (1 if fin else 0)) if mm else 0
                if ov <= capy:
                    m = mm
                    break
            if fin and m == 0 and cnt[127] >= th:
                s, gq, rl = per_core[c]
                s127 = s[(gq == g) & (rl == 127)]
                if (s127 < 0).all():
                    m = 1            # no nonneg idx to anchor the tail
            need = max(need, int(m))
        m_g.append(need)
    strip_of, nstrips = _strip_layout(m_g)

    cores = []
    for c in range(NCORES):
        s, g, rel = per_core[c]
        idx_cols = [np.zeros((128, 8), np.int16)]   # stub-gather zeros
        mask_cols = []
        nsa = max(nstrips, 1)
        st = np.zeros((128, nsa, 128), np.uint8)   # [slot-lane, strip, rel]
        ss = np.zeros((128, nsa, 128), np.uint8)   # [rel, strip, slot-lane]
        for pr in _pairs():
            seq_parts = []
            for gi, gg in enumerate(pr):
                th = th_g[gg]
                k = th - int(m_g[gg])
                seq = np.zeros(th * 128, np.int64)
                sel = g == gg
                es = s[sel]
                rl = rel[sel]
                order = np.argsort(rl, kind="stable")
                es, rl = es[order], rl[order]
                if gg == pr[-1] and int(m_g[gg]) == 0:
                    # anchor: make lane 127's last chosen diag edge nonneg
                    i127 = np.where(rl == 127)[0]
                    if len(i127) >= k:
                        chosen = i127[:k]
                        if es[chosen[-1]] < 0:
                            nn = i127[es[i127] >= 0]
                            assert len(nn), "unsafe tail not bumped"
                            a, b = chosen[-1], nn[-1]
                            es[a], es[b] = es[b], es[a]
                starts = np.searchsorted(rl, np.arange(128))
                occ = np.arange(len(rl)) - starts[rl]
                dsel = occ < k
                seq[occ[dsel] * 128 + rl[dsel]] = es[dsel]
                ov_es, ov_rl = es[~dsel], rl[~dsel]
                assert len(ov_es) <= max(128 * int(m_g[gg]) - 1, 0) or \
                    (int(m_g[gg]) and gg not in is_final
                     and len(ov_es) <= 128 * int(m_g[gg]))
                for j, (e_i, r_i) in enumerate(zip(ov_es, ov_rl)):
                    t, lane = k + j // 128, j % 128
                    seq[t * 128 + lane] = e_i
                    sidx = strip_of[(gg, j // 128)]
                    st[lane, sidx, r_i] = 1
                    ss[r_i, sidx, lane] = 1
                # p-mask: diag holes dead, general tiles pass-through
                cnt = cnts[c, gg]
                mcol = np.ones((128, th), np.float32)
                if k > 0:
                    mcol[:, :k] = (np.arange(k)[None, :]
                                   < np.minimum(cnt, k)[:, None])
                mask_cols.append(mcol)
                seq_parts.append(seq)
            idx_cols.append(_wrap_idx(np.concatenate(seq_parts)))
        cores.append(dict(
            idx=np.concatenate(idx_cols, axis=1),
            st=st.reshape(128, nsa * 128).astype(FP8),
            ss=ss.reshape(128, nsa * 128).astype(FP8),
            mask=np.concatenate(mask_cols, axis=1).astype(BF16),
        ))
    return cores, tuple(th_g), tuple(m_g)


def _node_major(arr, c):
    """[N, k] -> [128, G*k] f32 for core c's shard (zero-padded)."""
    k = arr.shape[1]
    out = np.zeros((G * 128, k), np.float32)
    out[:NSH] = arr[c * NSH:(c + 1) * NSH]
    return np.ascontiguousarray(
        out.reshape(G, 128, k).transpose(1, 0, 2).reshape(128, G * k))


def _dma_gather_thin(gp, out_ap, in_ap, idxs_ap, num_idxs, elem_size,
                     single_packet=False, queue_num=0):
    """BassGpSimd.dma_gather (HBM source, transpose=False, prepare=False)
    without the elem_size%256B assert: the SWDGE non-transpose ucode path
    supports arbitrary row payloads; only the source row STRIDE is 256B-
    granular (validated on hardware)."""
    gp._assert_queue_num(queue_num)
    assert idxs_ap.dtype == mybir.dt.int16
    assert in_ap.dtype == out_ap.dtype
    assert in_ap.space == MemorySpace.DRAM
    assert idxs_ap.space == MemorySpace.SBUF
    assert out_ap.space == MemorySpace.SBUF
    elem_step = in_ap.ap[0][0]
    assert ap_utils.ap_is_contiguous(out_ap.ap[1:])
    assert ap_utils.ap_is_contiguous(idxs_ap.ap[1:])
    assert in_ap.ap[-1][1] == out_ap.ap[-1][1] == elem_size
    assert out_ap.ap[0][1] * out_ap.ap[1][1] == round_up_to_multiple(num_idxs, 128)
    stride_bytes = elem_step * mybir.dt.size(in_ap.dtype)
    assert stride_bytes % 256 == 0 and stride_bytes // 256 < 256
    _in_ap = gp.lower_ap_dma(in_ap, for_custom_bir_dma=True)
    _idxs_ap = gp.lower_ap(idxs_ap)
    _out_ap = gp.lower_ap(out_ap)
    return gp.add_instruction(
        mybir.InstDMAGatherAnt(
            name=gp.bass.get_next_instruction_name(),
            ins=[*_in_ap, _idxs_ap, gp.lower_val_access(gp.to_reg(num_idxs))],
            outs=[_out_ap],
            transpose=False,
            num_idxs=num_idxs,
            elem_size=elem_size,
            stride_bytes_256=stride_bytes // 256,
            gen_mode=0,
            single_packet=single_packet,
            queue_num=queue_num,
            sbuf_tokens_per_rank=0,
            sbuf_free_dim_per_rank=0,
            sbuf_free_dim_pad_per_rank=0,
            sbuf_byte_offset=0,
        ))


def _build_program(th_g, m_g, skip_collectives=False):
    nc = bacc.Bacc("TRN2", target_bir_lowering=False, debug=False,
                   num_devices=NCORES)
    f32, bf16, fp8, i16 = _dt.float32, _dt.bfloat16, _dt.float8e4, _dt.int16
    strip_of, nstrips = _strip_layout(m_g)
    nsa = max(nstrips, 1)
    NTT = sum(th_g)              # total V tiles
    NTMAX = max(sum(th_g[gg] for gg in pr) for pr in _pairs())
    IDXC = 8 + NTT * 8           # 8 stub cols + tiles*128/16
    MC = NTT                     # mask cols, (pair, gi, t) order
    EC2 = OUT + 1                # thin layer-2 row: feat2 | el2

    table0 = nc.dram_tensor("table0", [N, 256], bf16, kind="ExternalInput")
    x_nd = nc.dram_tensor("x_nd", [128, G * 128], bf16, kind="ExternalInput")
    er0_in = nc.dram_tensor("er0_in", [128, G * 2], bf16, kind="ExternalInput")
    idx_in = nc.dram_tensor("idx_in", [128, IDXC], i16, kind="ExternalInput")
    st_in = nc.dram_tensor("st_in", [128, nsa * 128], fp8, kind="ExternalInput")
    ss_in = nc.dram_tensor("ss_in", [128, nsa * 128], fp8, kind="ExternalInput")
    mask_in = nc.dram_tensor("mask_in", [128, MC], bf16, kind="ExternalInput")
    w1_in = nc.dram_tensor("w1_in", [128, 128], bf16, kind="ExternalInput")
    wle1_in = nc.dram_tensor("wle1_in", [128, 4], bf16, kind="ExternalInput")
    b0_in = nc.dram_tensor("b0_in", [128, 128], f32, kind="ExternalInput")
    b1_in = nc.dram_tensor("b1_in", [128, 128], f32, kind="ExternalInput")
    w2_in = nc.dram_tensor("w2_in", [128, OUT], bf16, kind="ExternalInput")
    wle2_in = nc.dram_tensor("wle2_in", [128, 2], bf16, kind="ExternalInput")
    b2_in = nc.dram_tensor("b2_in", [128, OUT], f32, kind="ExternalInput")
    out_d = nc.dram_tensor("out_lsm", [NSH, OUT], f32, kind="ExternalOutput")

    # per-pair: (idx col offset, tile offset, strip base, strip count, ntiles)
    pair_info = []
    io, to = 8, 0
    for pr in _pairs():
        sb = min((strip_of[(gg, 0)] for gg in pr if m_g[gg] > 0), default=0)
        ns = sum(m_g[gg] for gg in pr)
        nt = sum(th_g[gg] for gg in pr)
        pair_info.append((io, to, sb, ns, nt))
        io += nt * 8
        to += nt

    with tile.TileContext(nc) as tc:
        nc.gpsimd.load_library(library_config.mlp)
        with (
            tc.tile_pool(name="const", bufs=1) as cp,
            tc.tile_pool(name="state", bufs=1) as sp,
            tc.tile_pool(name="vpool", bufs=6) as vp,
            tc.tile_pool(name="stream", bufs=4) as fp,
            tc.tile_pool(name="small", bufs=4) as mp,
            tc.tile_pool(name="psA", bufs=2, space="PSUM") as pA,
            tc.tile_pool(name="psC", bufs=3, space="PSUM") as pC,
            tc.tile_pool(name="psB", bufs=1, space="PSUM") as pB,
            tc.tile_pool(name="dram", bufs=1, space="DRAM") as dp,
        ):
            def const_tile(shape, dtype, src, tag):
                t = cp.tile(shape, dtype, tag=tag)
                nc.sync.dma_start(t[:], src[:])
                return t

            idx_sb = const_tile([128, IDXC], i16, idx_in, "c_idx")
            moff_sb = const_tile([128, MC], bf16, mask_in, "c_moff")
            xnd_sb = const_tile([128, G * 128], bf16, x_nd, "c_xnd")
            w1 = const_tile([128, 128], bf16, w1_in, "c_w1")
            wle1 = const_tile([128, 4], bf16, wle1_in, "c_wle1")
            b0c = const_tile([128, 128], f32, b0_in, "c_b0")
            b1c = const_tile([128, 128], f32, b1_in, "c_b1")
            w2 = const_tile([128, OUT], bf16, w2_in, "c_w2")
            wle2 = const_tile([128, 2], bf16, wle2_in, "c_wle2")
            b2c = const_tile([128, OUT], f32, b2_in, "c_b2")
            er0 = const_tile([128, G * 2], bf16, er0_in, "c_er0")
            ident = cp.tile([128, 128], bf16, tag="c_ident")
            make_identity(nc, ident[:])

            h1_nd = sp.tile([128, G * 128], bf16, tag="h1nd")
            h2_nd = sp.tile([128, G * 128], bf16, tag="h2nd")
            hfm = sp.tile([128, G * 128], bf16, tag="hfm")  # reused l1 -> l2
            h3 = sp.tile([128, G * OUT], f32, tag="h3")
            er1 = sp.tile([128, G * 2], bf16, tag="er1")
            er2 = sp.tile([128, G * 1], bf16, tag="er2")

            tsh1 = dp.tile([NSH, 256], bf16)
            tfull1 = dp.tile([N, 256], bf16)
            tsh2 = dp.tile([NSH, 128], bf16)
            tfull2 = dp.tile([N, 128], bf16)

            def edge_stage(layer, mid_cb=None, mid_at=0):
                if layer == 0:
                    table, rowc, nh, fdim = table0, 130, 2, 128
                    er_sb, res, bvec = er0, None, b0c
                elif layer == 1:
                    table, rowc, nh, fdim = tfull1, 130, 2, 128
                    er_sb, res, bvec = er1, h1_nd, b1c
                else:
                    table, rowc, nh, fdim = tfull2, EC2, 1, OUT
                    er_sb, res, bvec = er2, None, b2c
                vsc = fdim + nh
                hd = fdim // nh

                if layer > 0:
                    # ordering stub: Pool queue is in-order, so this gather's
                    # wait on the lo-half AllGather chunks orders every
                    # following (hi-based, signed) gather after them.
                    stub = fp.tile([128, 1, rowc], bf16, tag="vstub")
                    _dma_gather_thin(nc.gpsimd, stub[:],
                                     table[0:HALF, 0:rowc],
                                     idx_sb[:, 0:8], 128, rowc)

                for pi, pr in enumerate(_pairs()):
                    if mid_cb is not None and pi == mid_at:
                        mid_cb()
                    io, to, sb, ns, nt = pair_info[pi]

                    if ns:
                        st_sb = fp.tile([128, ns * 128], fp8, tag="st")
                        nc.sync.dma_start(
                            st_sb[:], st_in[:, sb * 128:(sb + ns) * 128])
                        ss_sb = fp.tile([128, ns * 128], fp8, tag="ss")
                        nc.sync.dma_start(
                            ss_sb[:], ss_in[:, sb * 128:(sb + ns) * 128])

                    v = vp.tile([128, NTMAX, rowc], bf16, tag="v")
                    ii = idx_sb[:, io:io + nt * 8]
                    _dma_gather_thin(nc.gpsimd, v[:, 0:nt, :],
                                     table[HALF:N, 0:rowc], ii,
                                     nt * 128, rowc)

                    tb_of = []
                    tacc = 0
                    for gg in pr:
                        tb_of.append(tacc)
                        tacc += th_g[gg]

                    # scores: el + er_dst (+ hole Moff); general overwritten
                    score = mp.tile([128, NTMAX * nh], f32, tag="score")
                    for gi, gg in enumerate(pr):
                        tb, th = tb_of[gi], th_g[gg]
                        nc.vector.tensor_tensor(
                            out=score[:, tb * nh:(tb + th) * nh]
                                .rearrange("p (t h) -> p t h", h=nh),
                            in0=v[:, tb:tb + th, fdim:fdim + nh],
                            in1=er_sb[:, gg * nh:(gg + 1) * nh]
                                .unsqueeze(1).to_broadcast([128, th, nh]),
                            op=_AL.add)
                    if ns:
                        er_ps = pA.tile([128, ns * nh], f32, space="PSUM",
                                        tag="erp")
                        for gi, gg in enumerate(pr):
                            for jj in range(m_g[gg]):
                                sl = strip_of[(gg, jj)] - sb
                                nc.tensor.matmul(
                                    out=er_ps[:, sl * nh:(sl + 1) * nh],
                                    lhsT=ss_sb[:, sl * 128:(sl + 1) * 128],
                                    rhs=er_sb[:, gg * nh:(gg + 1) * nh],
                                    start=True, stop=True)
                        for gi, gg in enumerate(pr):
                            ml = m_g[gg]
                            if not ml:
                                continue
                            sl = strip_of[(gg, 0)] - sb
                            tb = tb_of[gi] + th_g[gg] - ml
                            nc.vector.tensor_tensor(
                                out=score[:, tb * nh:(tb + ml) * nh]
                                    .rearrange("p (t h) -> p t h", h=nh),
                                in0=v[:, tb:tb + ml, fdim:fdim + nh],
                                in1=er_ps[:, sl * nh:(sl + ml) * nh]
                                    .rearrange("p (t h) -> p t h", h=nh),
                                op=_AL.add)
                    # p = exp(lrelu(s)) = max(exp(s), exp(0.2 s)), masked
                    pa_t = mp.tile([128, NTMAX * nh], f32, tag="pa")
                    pb_t = mp.tile([128, NTMAX * nh], f32, tag="pb")
                    nc.scalar.activation(pa_t[:, 0:nt * nh],
                                         score[:, 0:nt * nh], _AF.Exp)
                    nc.scalar.activation(pb_t[:, 0:nt * nh],
                                         score[:, 0:nt * nh], _AF.Exp,
                                         scale=NEG)
                    p = mp.tile([128, NTMAX * nh], bf16, tag="p")
                    nc.vector.tensor_max(p[:, 0:nt * nh], pa_t[:, 0:nt * nh],
                                         pb_t[:, 0:nt * nh])
                    pm = mp.tile([128, NTMAX * nh], bf16, tag="pm")
                    nc.vector.tensor_tensor(
                        out=pm[:, 0:nt * nh].rearrange("p (x h) -> p x h", h=nh),
                        in0=p[:, 0:nt * nh].rearrange("p (x h) -> p x h", h=nh),
                        in1=moff_sb[:, to:to + nt]
                            .unsqueeze(2).to_broadcast([128, nt, nh]),
                        op=_AL.mult)

                    vs = fp.tile([128, NTMAX, vsc], bf16, tag="vs")
                    nc.vector.tensor_tensor(
                        out=vs[:, 0:nt, 0:fdim]
                            .rearrange("p t (d h) -> p t d h", h=nh),
                        in0=v[:, 0:nt, 0:fdim]
                            .rearrange("p t (d h) -> p t d h", h=nh),
                        in1=pm[:, 0:nt * nh].rearrange("p (t h) -> p t h", h=nh)
                            .unsqueeze(2).to_broadcast([128, nt, hd, nh]),
                        op=_AL.mult)
                    nc.vector.tensor_copy(
                        vs[:, 0:nt, fdim:fdim + nh],
                        pm[:, 0:nt * nh].rearrange("p (t h) -> p t h", h=nh))

                    for gi, gg in enumerate(pr):
                        th = th_g[gg]
                        k = th - m_g[gg]
                        acc = pC.tile([128, vsc], f32, space="PSUM", tag="acc")
                        for t in range(th):
                            if t < k:
                                lhsT = ident[:]
                            else:
                                sl = strip_of[(gg, t - k)] - sb
                                lhsT = st_sb[:, sl * 128:(sl + 1) * 128]
                            nc.tensor.matmul(
                                out=acc[:],
                                lhsT=lhsT,
                                rhs=vs[:, tb_of[gi] + t, :],
                                start=(t == 0), stop=(t == th - 1))
                        ssb = mp.tile([128, nh], f32, tag="ssb")
                        nc.vector.tensor_scalar(
                            ssb[:], acc[:, fdim:fdim + nh], 1e-30, None, _AL.max)
                        rs = mp.tile([128, nh], f32, tag="rs")
                        nc.vector.reciprocal(rs[:], ssb[:])
                        o = mp.tile([128, fdim], f32, tag="o")
                        ov = o[:].rearrange("p (d h) -> p d h", h=nh)
                        av = acc[:, 0:fdim].rearrange("p (d h) -> p d h", h=nh)
                        for h in range(nh):
                            nc.scalar.activation(
                                ov[:, :, h:h + 1], av[:, :, h:h + 1],
                                _AF.Copy, scale=rs[:, h:h + 1])
                        if layer == 2:
                            nc.vector.tensor_add(
                                h3[:, gg * OUT:(gg + 1) * OUT], o[:], b2c[:])
                            continue
                        xb = mp.tile([128, fdim], f32, tag="xb")
                        nc.vector.tensor_add(xb[:], o[:], bvec[:])
                        # elu(x) = max(x,0) + min(exp(min(x,0)) - 1, 0)
                        t1 = mp.tile([128, fdim], f32, tag="t1")
                        nc.vector.tensor_scalar_min(t1[:], xb[:], 0.0)
                        e1 = mp.tile([128, fdim], f32, tag="e1")
                        nc.scalar.activation(e1[:], t1[:], _AF.Exp)
                        t2 = mp.tile([128, fdim], f32, tag="t2")
                        nc.vector.tensor_scalar(
                            t2[:], e1[:], -1.0, 0.0, _AL.add, _AL.min)
                        t3 = mp.tile([128, fdim], f32, tag="t3")
                        nc.vector.tensor_scalar_max(t3[:], xb[:], 0.0)
                        elu = mp.tile([128, fdim], f32, tag="elu")
                        nc.vector.tensor_add(elu[:], t2[:], t3[:])
                        h_nd = h1_nd if layer == 0 else h2_nd
                        hsl = h_nd[:, gg * 128:(gg + 1) * 128]
                        if layer == 0:
                            nc.vector.tensor_add(
                                hsl, elu[:], xnd_sb[:, gg * 128:(gg + 1) * 128])
                        else:
                            nc.vector.tensor_add(
                                hsl, elu[:], res[:, gg * 128:(gg + 1) * 128])
                        # next layer's node stage, fused
                        tp = pB.tile([128, 128], bf16, space="PSUM", tag="tp")
                        nc.tensor.transpose(out=tp[:], in_=hsl, identity=ident[:])
                        fsl = hfm[:, gg * 128:(gg + 1) * 128]
                        nc.scalar.copy(fsl, tp[:])
                        wn = w1 if layer == 0 else w2
                        wlen = wle1 if layer == 0 else wle2
                        fnext = 128 if layer == 0 else OUT
                        nhn = 2 if layer == 0 else 1
                        featp = pB.tile([128, fnext], f32, space="PSUM", tag="featp")
                        nc.tensor.matmul(out=featp[:], lhsT=fsl, rhs=wn[:],
                                         start=True, stop=True)
                        elp = pB.tile([128, 2 * nhn], f32, space="PSUM", tag="elp")
                        nc.tensor.matmul(out=elp[:], lhsT=fsl, rhs=wlen[:],
                                         start=True, stop=True)
                        wcol = fnext + nhn
                        tt = mp.tile([128, wcol], bf16, tag="ttile")
                        nc.scalar.copy(tt[:, 0:fnext], featp[:])
                        nc.vector.tensor_copy(
                            tt[:, fnext:fnext + nhn], elp[:, 0:nhn])
                        ern = er1 if layer == 0 else er2
                        nc.vector.tensor_copy(
                            ern[:, gg * nhn:(gg + 1) * nhn], elp[:, nhn:2 * nhn])
                        tshn = tsh1 if layer == 0 else tsh2
                        nrows = min(128, NSH - gg * 128)
                        nc.sync.dma_start(
                            tshn[gg * 128:gg * 128 + nrows, 0:wcol],
                            tt[:nrows, :])

            def chunked_allgather(tsh, tfull):
                # per-chunk collectives overlap halo exchange with the
                # producing layer's tail groups; table rows are permuted on
                # the host (_new_row) so each chunk's output is contiguous
                bounds = _chunk_bounds()
                for lo, hi in zip(bounds[:-1], bounds[1:]):
                    nc.gpsimd.collective_compute(
                        "AllGather", _AL.bypass,
                        replica_groups=[list(range(NCORES))],
                        ins=[tsh[lo:hi, :].opt()],
                        outs=[tfull[NCORES * lo:NCORES * hi, :].opt()])

            # log_softmax over classes: x - ln(sum exp(x)), done in two
            # group ranges so the first half overlaps layer 2's tail pairs
            ex = sp.tile([128, G * OUT], f32, tag="lsx")
            sm = sp.tile([128, G], f32, tag="lss")
            ls = sp.tile([128, G], f32, tag="lsl")
            nfull = NSH // 128
            rem = NSH - nfull * 128

            def log_softmax_part(gl, gh):
                ng = gh - gl
                exs = ex[:, gl * OUT:gh * OUT]
                nc.scalar.activation(exs, h3[:, gl * OUT:gh * OUT], _AF.Exp)
                nc.vector.tensor_reduce(
                    sm[:, gl:gh], exs.rearrange("p (g c) -> p g c", c=OUT),
                    axis=mybir.AxisListType.X, op=_AL.add)
                nc.scalar.activation(ls[:, gl:gh], sm[:, gl:gh], _AF.Ln)
                lsm = exs  # reuse the exp scratch for the output
                nc.vector.tensor_tensor(
                    out=lsm.rearrange("p (g c) -> p g c", c=OUT),
                    in0=h3[:, gl * OUT:gh * OUT]
                        .rearrange("p (g c) -> p g c", c=OUT),
                    in1=ls[:, gl:gh].unsqueeze(2).to_broadcast([128, ng, OUT]),
                    op=_AL.subtract)
                gfull = min(gh, nfull)
                if gfull > gl:
                    nc.sync.dma_start(
                        out_d[gl * 128:gfull * 128, :]
                            .rearrange("(g p) c -> p g c", p=128),
                        lsm[:, 0:(gfull - gl) * OUT]
                            .rearrange("p (g c) -> p g c", c=OUT))
                if gh > nfull and rem:
                    nc.sync.dma_start(
                        out_d[nfull * 128:NSH, :],
                        lsm[0:rem, (nfull - gl) * OUT:(nfull - gl + 1) * OUT])

            GSPLIT = 26

            edge_stage(0)
            if not skip_collectives:
                chunked_allgather(tsh1, tfull1)
            edge_stage(1)
            if not skip_collectives:
                chunked_allgather(tsh2, tfull2)
            edge_stage(2, mid_cb=lambda: log_softmax_part(0, GSPLIT),
                       mid_at=GSPLIT // 2 + 2)
            log_softmax_part(GSPLIT, G)

    nc.compile()
    return nc


_PROG_CACHE = {}


def kernel(x, src, dst, W0, al0, ar0, b0, W1, al1, ar1, b1,
           W2, al2, ar2, b2, trace=False):
    x = np.asarray(x, np.float32)
    src = np.asarray(src).astype(np.int64)
    dst = np.asarray(dst).astype(np.int64)
    W0, al0, ar0, b0 = (np.asarray(a, np.float32) for a in (W0, al0, ar0, b0))
    W1, al1, ar1, b1 = (np.asarray(a, np.float32) for a in (W1, al1, ar1, b1))
    W2, al2, ar2, b2 = (np.asarray(a, np.float32) for a in (W2, al2, ar2, b2))

    cores, th_g, m_g = _preprocess(src, dst)

    # head-interleaved feature order: new col j=(d,h) <- orig col h*64+d.
    # Makes the per-edge attention scale broadcast innermost-stride-1 on DVE.
    PERM = np.array([(j % 2) * HID + j // 2 for j in range(128)])

    # host layer-0 node stage
    feat0 = (x @ W0).reshape(N, 2, HID)
    el0 = np.einsum("nhd,hd->nh", feat0, al0).astype(np.float32)
    er0 = np.einsum("nhd,hd->nh", feat0, ar0).astype(np.float32)
    table0 = np.zeros((N, 256), BF16)
    nr = _new_row()
    table0[nr, 0:128] = feat0.reshape(N, 128)[:, PERM].astype(BF16)
    table0[nr, 128:130] = el0.astype(BF16)

    W1p = W1[PERM, :]              # rows: h1 arrives interleaved
    wle1 = np.zeros((128, 4), np.float32)
    for h in range(2):
        wle1[:, h] = W1p[:, h * HID:(h + 1) * HID] @ al1[h]
        wle1[:, 2 + h] = W1p[:, h * HID:(h + 1) * HID] @ ar1[h]
    W1pi = W1p[:, PERM]            # cols: feat1 comes out interleaved
    W2p = W2[PERM, :]
    wle2 = np.zeros((128, 2), np.float32)
    wle2[:, 0] = W2p @ al2[0]
    wle2[:, 1] = W2p @ ar2[0]

    key = (th_g, m_g)
    if key not in _PROG_CACHE:
        _PROG_CACHE[key] = _build_program(th_g, m_g)
    nc = _PROG_CACHE[key]

    in_maps = []
    for c in range(NCORES):
        cc = cores[c]
        in_maps.append(dict(
            table0=table0,
            x_nd=_node_major(x[:, PERM], c).astype(BF16),
            er0_in=_node_major(er0, c).astype(BF16),
            idx_in=cc["idx"],
            st_in=cc["st"],
            ss_in=cc["ss"],
            mask_in=cc["mask"],
            w1_in=W1pi.astype(BF16),
            wle1_in=wle1.astype(BF16),
            b0_in=np.tile(b0[None, PERM], (128, 1)).astype(np.float32),
            b1_in=np.tile(b1[None, PERM], (128, 1)).astype(np.float32),
            w2_in=W2p.astype(BF16),
            wle2_in=wle2.astype(BF16),
            b2_in=np.tile(b2[None, :], (128, 1)).astype(np.float32),
        ))
    def run_once():
        res = run_bass_kernel_spmd(nc, in_maps, core_ids=list(range(NCORES)),
                                   trace=trace)
        kernel._last_result = res
        return np.concatenate(
            [res.results[c]["out_lsm"] for c in range(NCORES)],
            axis=0).astype(np.float32)

    # the very first dispatch after NEFF load has been observed (rarely) to
    # race device init; execute twice and require agreement, arbitrating
    # with a third run on mismatch
    out = run_once()
    out2 = run_once()
    if not np.allclose(out, out2, atol=1e-3, rtol=1e-2):
        out3 = run_once()
        out = out3 if np.allclose(out2, out3, atol=1e-3, rtol=1e-2) else out2
    return out
